# revision 1
# baseline (speedup 1.0000x reference)
"""Trainium2 Bass kernel for nn_MultiHeadAttention_71502615544564 (GNN
message-passing multi-head attention).

Math note: the reference computes
    out = segment_sum(v[dst] * attn_weights[..., None], dst)
Because v is indexed by the same dst as the segment reduction,
    out[n] = v[n] * (sum_e exp_attn[e]) / (sum_exp[n] + 1e-8)
           = v[n] * s_n / (s_n + 1e-8).
Any relative error r in s_n perturbs the output by <= (1e-8 / s_n) * r
(~1e-7 absolute), so the attention/exp/scatter pipeline only needs rough
precision; exact f32 is only required for the V projection and the output
matmul. The global per-head max subtraction is likewise a no-op up to
~1e-9 in the output (exp never overflows for this data), so it is dropped.

Sharding: edges are assigned to the core that owns dst (8 node ranges of
6250).  k-gather, the sum_exp scatter and the output stage are then fully
core-local; only q[src] needs the full (replicated) q table.
"""

import sys

sys.path.insert(0, "/opt/trn_rl_repo")

import ml_dtypes
import numpy as np

import concourse.bacc as bacc
import concourse.mybir as mybir
import concourse.tile as tile
from concourse.bass_utils import run_bass_kernel_spmd

P = 128
N, DIM, H, HD = 50000, 128, 8, 16
E = 640000
NCORES = 8
NLOC = N // NCORES            # 6250
NQT = (N + P - 1) // P        # 391 q tiles
NQR = NQT * P                 # 50048 q-table rows
NKC = (NLOC + P - 1) // P     # 49 local cols per partition
NKR = NKC * P                 # 6272 local rows
GARBAGE = 106 * NKC + 48      # swizzled row unused by any real node (=5242)
SPLIT = 32768                 # int16 positive range split for q gather
CH = 8192                     # edge chunk size
SUMW = 64                     # sum-table row width (f32) -> 256B stride
EXP_SCALE = 1.0 / float(HD) ** 0.5   # exp(attn * 1/sqrt(hd))

F32 = mybir.dt.float32
BF16 = mybir.dt.bfloat16
I16 = mybir.dt.int16
BF = ml_dtypes.bfloat16


def _wrap_idx(a):
    """int16 position-wrapped index stream: pos i -> [i%16, i//16],
    replicated across the 8 GPSIMD 16-partition groups -> [128, len/16]."""
    assert len(a) % 16 == 0
    base = np.ascontiguousarray(a.reshape(-1, 16).T)
    return np.tile(base, (8, 1))


def _chunks(total, grp):
    out = []
    off = 0
    while off < total:
        sz = min(CH, total - off)
        out.append((off, sz, grp))
        off += sz
    return out


def build_program(LA, LB, phases="ABC"):
    """One SPMD program; LA/LB are the (core-uniform) padded edge counts of
    the two q-index ranges."""
    LP = LA + LB
    chunks = _chunks(LA, 0) + [(LA + o, sz, g + 1) for o, sz, g in _chunks(LB, 0)]

    nc = bacc.Bacc("TRN2", target_bir_lowering=False, debug=False)

    xT = nc.dram_tensor("xT", [P, NQR], BF16, kind="ExternalInput")
    xlocT = nc.dram_tensor("xlocT", [P, NKR], F32, kind="ExternalInput")
    qidx = nc.dram_tensor("qidx", [P, LP // 16], I16, kind="ExternalInput")
    sidx = nc.dram_tensor("sidx", [P, LP // 16], I16, kind="ExternalInput")
    wq = nc.dram_tensor("wq", [DIM, DIM], BF16, kind="ExternalInput")
    bq = nc.dram_tensor("bq", [1, DIM], BF16, kind="ExternalInput")
    wk = nc.dram_tensor("wk", [DIM, DIM], F32, kind="ExternalInput")
    bk = nc.dram_tensor("bk", [1, DIM], F32, kind="ExternalInput")
    wv = nc.dram_tensor("wv", [DIM, DIM], F32, kind="ExternalInput")
    bv = nc.dram_tensor("bv", [1, DIM], F32, kind="ExternalInput")
    wout = nc.dram_tensor("wout", [DIM, DIM], F32, kind="ExternalInput")
    bout = nc.dram_tensor("bout", [1, DIM], F32, kind="ExternalInput")
    emat = nc.dram_tensor("emat", [H, DIM], F32, kind="ExternalInput")

    q_table = nc.dram_tensor("q_table", [NQR, DIM], BF16)
    k_table = nc.dram_tensor("k_table", [NKR, DIM], BF16)
    sum_table = nc.dram_tensor("sum_table", [NKR, SUMW], F32)

    out_loc = nc.dram_tensor("out_loc", [P, NKC, DIM], F32, kind="ExternalOutput")

    from concourse.masks import make_identity

    with tile.TileContext(nc) as tc:
        with (
            tc.tile_pool(name="const", bufs=1) as cpool,
            tc.tile_pool(name="persist", bufs=1) as pers,
        ):
            # ---- constants ----
            wq_sb = cpool.tile([DIM, DIM], BF16)
            nc.sync.dma_start(out=wq_sb[:], in_=wq[:])
            bq_sb = cpool.tile([1, DIM], BF16)
            nc.sync.dma_start(out=bq_sb[:], in_=bq[:])
            wk_sb = cpool.tile([DIM, DIM], F32)
            nc.sync.dma_start(out=wk_sb[:], in_=wk[:])
            bk_sb = cpool.tile([1, DIM], F32)
            nc.sync.dma_start(out=bk_sb[:], in_=bk[:])
            wv_sb = cpool.tile([DIM, DIM], F32)
            nc.sync.dma_start(out=wv_sb[:], in_=wv[:])
            bv_sb = cpool.tile([1, DIM], F32)
            nc.sync.dma_start(out=bv_sb[:], in_=bv[:])
            wo_sb = cpool.tile([DIM, DIM], F32)
            nc.sync.dma_start(out=wo_sb[:], in_=wout[:])
            bo_sb = cpool.tile([1, DIM], F32)
            nc.sync.dma_start(out=bo_sb[:], in_=bout[:])
            em_sb = cpool.tile([H, DIM], F32)
            nc.sync.dma_start(out=em_sb[:], in_=emat[:])
            ones_bf = cpool.tile([1, DIM], BF16)
            nc.vector.memset(ones_bf[:], 1.0)
            ones_f = cpool.tile([1, 512], F32)
            nc.vector.memset(ones_f[:], 1.0)
            ident = cpool.tile([P, P], F32)
            make_identity(nc, ident[:])

            # persistent buffers
            vT_sb = pers.tile([P, NKR], F32)           # v transposed [o, n]
            exp_sb = pers.tile([P, LP // P, H], F32)   # per-edge exp values
            qidx_sb = pers.tile([P, LP // 16], I16)
            sidx_sb = pers.tile([P, LP // 16], I16)
            nc.sync.dma_start(out=qidx_sb[:], in_=qidx[:])
            nc.sync.dma_start(out=sidx_sb[:], in_=sidx[:])

            st_flat = sum_table[:].rearrange("(p c) w -> p (c w)", p=P)

            # ---- Phase A: q table (bf16), k table (bf16), vT (f32) ----
            # q: lhsT = xT block [i, n], rhs = Wq -> psum [n, o]
            pA_cm = tc.tile_pool(name="phaseA", bufs=1)
            xpool_cm = tc.tile_pool(name="xstream", bufs=3)
            qbpool_cm = tc.tile_pool(name="qbatch", bufs=2)
            psA_cm = tc.tile_pool(name="psA", bufs=2, space="PSUM")
            pA = pA_cm.__enter__()
            xpool = xpool_cm.__enter__()
            qbpool = qbpool_cm.__enter__()
            psA = psA_cm.__enter__()
            QB = 8  # q tiles per table-write batch
            for t0 in range(0, NQT, QB):
                nb = min(QB, NQT - t0)
                qb_sb = qbpool.tile([P, QB, DIM], BF16, tag="qb")
                for j in range(nb):
                    t = t0 + j
                    xt = xpool.tile([P, P], BF16, tag="xt")
                    nc.sync.dma_start(out=xt[:], in_=xT[:, t * P:(t + 1) * P])
                    qp = psA.tile([P, DIM], F32, tag="qp")
                    nc.tensor.matmul(out=qp[:], lhsT=xt[:], rhs=wq_sb[:],
                                     start=True, stop=False)
                    nc.tensor.matmul(out=qp[:], lhsT=ones_bf[:], rhs=bq_sb[:],
                                     start=False, stop=True)
                    nc.vector.tensor_copy(out=qb_sb[:, j, :], in_=qp[:])
                # swizzled q rows: row (p*NQT + t) <-> node 128t+p
                qv = q_table[:].rearrange("(p c) d -> p c d", p=P)
                nc.sync.dma_start(out=qv[:, t0:t0 + nb, :], in_=qb_sb[:, :nb, :])

            # zero the sum table (swizzled view: row p*NKC+c <-> [p, c])
            zt = pA.tile([P, NKC * SUMW], F32)
            nc.vector.memset(zt[:], 0.0)
            nc.sync.dma_start(out=st_flat, in_=zt[:])

            # k & vT from xlocT
            xl_sb = pA.tile([P, NKR], F32)
            nc.sync.dma_start(out=xl_sb[:], in_=xlocT[:])
            k_sb = pA.tile([P, NKC, DIM], BF16)
            for t in range(NKC):
                kp = psA.tile([P, DIM], F32, tag="kp")
                nc.tensor.matmul(out=kp[:], lhsT=xl_sb[:, t * P:(t + 1) * P],
                                 rhs=wk_sb[:], start=True, stop=False)
                nc.tensor.matmul(out=kp[:], lhsT=ones_f[:, :P], rhs=bk_sb[:],
                                 start=False, stop=True)
                nc.vector.tensor_copy(out=k_sb[:, t, :], in_=kp[:])
            kv = k_table[:].rearrange("(p c) d -> p c d", p=P)
            nc.sync.dma_start(out=kv[:], in_=k_sb[:])

            for b0 in range(0, NKR, 512):
                nb = min(512, NKR - b0)
                vp = psA.tile([P, 512], F32, tag="vp")
                nc.tensor.matmul(out=vp[:, :nb], lhsT=wv_sb[:],
                                 rhs=xl_sb[:, b0:b0 + nb], start=True, stop=False)
                nc.tensor.matmul(out=vp[:, :nb], lhsT=bv_sb[:],
                                 rhs=ones_f[:, :nb], start=False, stop=True)
                nc.vector.tensor_copy(out=vT_sb[:, b0:b0 + nb], in_=vp[:, :nb])

            psA_cm.__exit__(None, None, None)
            qbpool_cm.__exit__(None, None, None)
            xpool_cm.__exit__(None, None, None)
            pA_cm.__exit__(None, None, None)

            # ---- Phase B: gather q/k rows per edge, dot, exp, scatter ----
            gpool_cm = tc.tile_pool(name="gath", bufs=2)
            wpool_cm = tc.tile_pool(name="work", bufs=2)
            gpool = gpool_cm.__enter__()
            wpool = wpool_cm.__enter__()
            blvl = 9
            for ph in phases.split(","):
                if ph.startswith("B") and len(ph) > 1:
                    blvl = int(ph[1])
            if blvl == 6:
                nc.vector.memset(exp_sb[:], 1.0)
            for off, sz, grp in (chunks if "B" in phases else []):
                if blvl == 6:
                    for so in range(off, off + sz, 4096):
                        ssz = min(4096, off + sz - so)
                        nc.gpsimd.dma_scatter_add(
                            out_ap=sum_table[:, :H],
                            in_ap=exp_sb[:, so // P:(so + ssz) // P, :],
                            idxs_ap=sidx_sb[:, so // 16:(so + ssz) // 16],
                            num_idxs=ssz, num_idxs_reg=ssz,
                            elem_size=H, elem_step=SUMW, single_packet=False)
                    continue
                qc = gpool.tile([P, CH // P, DIM], BF16, tag="qc")
                src_ap = q_table[:] if grp == 0 else q_table[SPLIT:NQR, :]
                nc.gpsimd.dma_gather(
                    out_ap=qc[:, :sz // P, :], in_ap=src_ap,
                    idxs_ap=qidx_sb[:, off // 16:(off + sz) // 16],
                    num_idxs=sz, num_idxs_reg=sz, elem_size=DIM,
                    single_packet=False)
                kc = gpool.tile([P, CH // P, DIM], BF16, tag="kc")
                nc.gpsimd.dma_gather(
                    out_ap=kc[:, :sz // P, :], in_ap=k_table[:],
                    idxs_ap=sidx_sb[:, off // 16:(off + sz) // 16],
                    num_idxs=sz, num_idxs_reg=sz, elem_size=DIM,
                    single_packet=False)
                if blvl < 2:
                    continue
                prod = wpool.tile([P, CH // P, DIM], BF16, tag="prod")
                nc.vector.tensor_tensor(out=prod[:, :sz // P, :],
                                        in0=qc[:, :sz // P, :],
                                        in1=kc[:, :sz // P, :],
                                        op=mybir.AluOpType.mult)
                if blvl < 3:
                    continue
                attn = wpool.tile([P, CH // P, H], F32, tag="attn")
                nc.vector.tensor_reduce(
                    out=attn[:, :sz // P, :],
                    in_=prod[:, :sz // P, :].rearrange("p b (h d) -> p b h d", d=HD),
                    axis=mybir.AxisListType.X, op=mybir.AluOpType.add)
                if blvl < 4:
                    continue
                nc.scalar.activation(
                    out=exp_sb[:, off // P:(off + sz) // P, :],
                    in_=attn[:, :sz // P, :],
                    func=mybir.ActivationFunctionType.Exp, scale=EXP_SCALE)
                if blvl < 5:
                    continue
                for so in range(off, off + sz, 4096):
                    ssz = min(4096, off + sz - so)
                    nc.gpsimd.dma_scatter_add(
                        out_ap=sum_table[:, :H],
                        in_ap=exp_sb[:, so // P:(so + ssz) // P, :],
                        idxs_ap=sidx_sb[:, so // 16:(so + ssz) // 16],
                        num_idxs=ssz, num_idxs_reg=ssz,
                        elem_size=H, elem_step=SUMW, single_packet=False)

            wpool_cm.__exit__(None, None, None)
            gpool_cm.__exit__(None, None, None)

            # ---- Phase C: ratio -> scale vT -> output matmul ----
            if "C" not in phases:
                dummy = pers.tile([P, NKC, DIM], F32)
                nc.vector.memset(dummy[:], 0.0)
                nc.sync.dma_start(out=out_loc[:], in_=dummy[:])
            else:
                pC_cm = tc.tile_pool(name="phaseC", bufs=1)
                psC_cm = tc.tile_pool(name="psC", bufs=2, space="PSUM")
                psB_cm = tc.tile_pool(name="psB", bufs=2, space="PSUM")
                pC = pC_cm.__enter__()
                psC = psC_cm.__enter__()
                psB = psB_cm.__enter__()
                sum_sb = pC.tile([P, NKC * SUMW], F32)
                nc.sync.dma_start(out=sum_sb[:], in_=st_flat)
                sview = sum_sb[:].rearrange("p (c w) -> p c w", w=SUMW)[:, :, 0:H]
                splus = pC.tile([P, NKC, H], F32)
                nc.vector.tensor_scalar(out=splus[:], in0=sview, scalar1=1e-8,
                                        scalar2=None, op0=mybir.AluOpType.add)
                recip = pC.tile([P, NKC, H], F32)
                nc.vector.reciprocal(out=recip[:], in_=splus[:])
                ratio = pC.tile([P, NKC, H], F32)
                nc.vector.tensor_tensor(out=ratio[:], in0=sview, in1=recip[:],
                                        op=mybir.AluOpType.mult)
                # transpose ratio -> [h, n] (n = c*128 + p)
                ratioT = pC.tile([H, NKC, P], F32)
                for c in range(NKC):
                    rp = psB.tile([H, P], F32, tag="rp")
                    nc.tensor.transpose(out=rp[:], in_=ratio[:, c, :], identity=ident[:])
                    nc.vector.tensor_copy(out=ratioT[:, c, :], in_=rp[:])
                # svT = vT * expand(ratio) ; expand via E matmul [8,128]^T
                svT = pC.tile([P, NKR], F32)
                for b0 in range(0, NKR, 512):
                    nb = min(512, NKR - b0)
                    rx = psB.tile([P, 512], F32, tag="rx")
                    nc.tensor.matmul(out=rx[:, :nb], lhsT=em_sb[:],
                                     rhs=ratioT[:].rearrange("h c p -> h (c p)")[:, b0:b0 + nb],
                                     start=True, stop=True)
                    nc.vector.tensor_tensor(out=svT[:, b0:b0 + nb],
                                            in0=vT_sb[:, b0:b0 + nb],
                                            in1=rx[:, :nb], op=mybir.AluOpType.mult)
                # out[n, o] = svT[:, n].T @ wout + bout
                out_sb = pC.tile([P, NKC, DIM], F32)
                for t in range(NKC):
                    op_ = psC.tile([P, DIM], F32, tag="op")
                    nc.tensor.matmul(out=op_[:], lhsT=svT[:, t * P:(t + 1) * P],
                                     rhs=wo_sb[:], start=True, stop=False)
                    nc.tensor.matmul(out=op_[:], lhsT=ones_f[:, :P], rhs=bo_sb[:],
                                     start=False, stop=True)
                    nc.vector.tensor_copy(out=out_sb[:, t, :], in_=op_[:])
                nc.sync.dma_start(out=out_loc[:], in_=out_sb[:])
                psB_cm.__exit__(None, None, None)
                psC_cm.__exit__(None, None, None)
                pC_cm.__exit__(None, None, None)

    nc.compile()
    return nc


def _prep(x, edge_index, W_qkv, b_qkv, W_out, b_out):
    x = np.asarray(x, np.float32)
    ei = np.asarray(edge_index, np.int64)
    W_qkv = np.asarray(W_qkv, np.float32)
    b_qkv = np.asarray(b_qkv, np.float32)
    W_out = np.asarray(W_out, np.float32)
    b_out = np.asarray(b_out, np.float32)

    src, dst = ei[0], ei[1]
    owner = dst // NLOC
    order = np.argsort(owner, kind="stable")
    counts = np.bincount(owner, minlength=NCORES)
    offs = np.zeros(NCORES + 1, np.int64)
    offs[1:] = np.cumsum(counts)

    # per-head column regrouping of the qkv projection
    hh = np.arange(H)[:, None]
    dd = np.arange(HD)[None, :]
    cols_q = (hh * 3 * HD + dd).ravel()
    cols_k = (hh * 3 * HD + HD + dd).ravel()
    cols_v = (hh * 3 * HD + 2 * HD + dd).ravel()

    per_core = []
    LA = LB = 0
    for c in range(NCORES):
        e = order[offs[c]:offs[c + 1]]
        s = src[e]
        d = dst[e] - c * NLOC
        qsw = (s % P) * NQT + s // P           # swizzled q row
        ssw = (d % P) * NKC + d // P           # swizzled local row
        a = qsw < SPLIT
        per_core.append((qsw[a], ssw[a], qsw[~a] - SPLIT, ssw[~a]))
        LA = max(LA, int(a.sum()))
        LB = max(LB, int((~a).sum()))
    LA = -(-LA // P) * P
    LB = -(-LB // P) * P

    in_maps = []
    xT_bf = np.zeros((P, NQR), BF)
    xT_bf[:, :N] = x.T.astype(BF)
    common = {
        "xT": xT_bf,
        "wq": W_qkv[:, cols_q].astype(BF),
        "bq": b_qkv[cols_q].astype(BF).reshape(1, DIM),
        "wk": W_qkv[:, cols_k].copy(),
        "bk": b_qkv[cols_k].reshape(1, DIM).copy(),
        "wv": W_qkv[:, cols_v].copy(),
        "bv": b_qkv[cols_v].reshape(1, DIM).copy(),
        "wout": W_out,
        "bout": b_out.reshape(1, DIM).copy(),
        "emat": np.repeat(np.eye(H, dtype=np.float32), HD, axis=1),
    }
    for c in range(NCORES):
        qa, sa, qb, sb = per_core[c]
        qi = np.zeros(LA + LB, np.int16)
        si = np.full(LA + LB, GARBAGE, np.int16)
        qi[:len(qa)] = qa
        si[:len(sa)] = sa
        qi[LA:LA + len(qb)] = qb
        si[LA:LA + len(sb)] = sb
        xl = np.zeros((P, NKR), np.float32)
        xl[:, :NLOC] = x[c * NLOC:(c + 1) * NLOC].T
        in_maps.append({
            **common,
            "xlocT": xl,
            "qidx": _wrap_idx(qi),
            "sidx": _wrap_idx(si),
        })
    return in_maps, LA, LB


_PROG_CACHE = {}
TRACE = False
LAST_RESULT = None
PHASES = "ABC"


def _install_ntff_hook():
    """Provide antenv.axon_hooks (absent in this image) so
    run_bass_kernel_spmd(trace=True) can NTFF-profile via libaxon."""
    import contextlib
    import ctypes
    import types

    if "antenv.axon_hooks" in sys.modules:
        return
    try:
        from antenv import axon_hooks  # noqa: F401
        return
    except ImportError:
        pass
    so_path = "/opt/axon/libaxon_pjrt.so"
    try:
        lib = ctypes.CDLL(so_path)
    except OSError:
        return
    if not hasattr(lib, "axon_start_nrt_profile"):
        return
    lib.axon_start_nrt_profile.argtypes = [
        ctypes.POINTER(ctypes.c_int64), ctypes.c_size_t]
    lib.axon_start_nrt_profile.restype = ctypes.c_int64
    lib.axon_stop_nrt_profile.argtypes = [ctypes.c_char_p]
    lib.axon_stop_nrt_profile.restype = ctypes.c_int64

    @contextlib.contextmanager
    def _hook(output_dir, device_ids):
        import jax
        jax.devices()
        if device_ids:
            ids = (ctypes.c_int64 * len(device_ids))(*device_ids)
            rc = lib.axon_start_nrt_profile(ids, len(device_ids))
        else:
            rc = lib.axon_start_nrt_profile(None, 0)
        if rc != 0:
            raise RuntimeError(f"axon_start_nrt_profile rc={rc}")
        try:
            yield
        finally:
            n = lib.axon_stop_nrt_profile(str(output_dir).encode())
            print(f"ntff profile: {n} file(s) -> {output_dir}", file=sys.stderr)

    _h = [_hook]
    m = types.ModuleType("antenv.axon_hooks")
    m.get_axon_ntff_profile_hook = lambda: _h[0]
    m.set_axon_ntff_profile_hook = lambda h: _h.__setitem__(0, h)
    sys.modules["antenv.axon_hooks"] = m
    import antenv
    antenv.axon_hooks = m


def kernel(x, edge_index, W_qkv, b_qkv, W_out, b_out):
    in_maps, LA, LB = _prep(x, edge_index, W_qkv, b_qkv, W_out, b_out)
    key = (LA, LB, PHASES)
    if key not in _PROG_CACHE:
        _PROG_CACHE[key] = build_program(LA, LB, PHASES)
    nc = _PROG_CACHE[key]
    if TRACE:
        _install_ntff_hook()
    res = run_bass_kernel_spmd(nc, in_maps, list(range(NCORES)), trace=TRACE)
    global LAST_RESULT
    LAST_RESULT = res
    out = np.empty((N, DIM), np.float32)
    ln = np.arange(NLOC)
    pp, cc = ln % P, ln // P
    for c in range(NCORES):
        o = np.asarray(res.results[c]["out_loc"])
        out[c * NLOC:(c + 1) * NLOC] = o[pp, cc, :]
    return out


if __name__ == "__main__":
    rng = np.random.default_rng(0)
    x = rng.standard_normal((N, DIM)).astype(np.float32)
    ei = rng.integers(0, N, (2, E)).astype(np.int64)
    lim = 1.0 / np.sqrt(DIM)
    W_qkv = rng.uniform(-lim, lim, (DIM, 3 * DIM)).astype(np.float32)
    b_qkv = rng.uniform(-lim, lim, (3 * DIM,)).astype(np.float32)
    W_out = rng.uniform(-lim, lim, (DIM, DIM)).astype(np.float32)
    b_out = rng.uniform(-lim, lim, (DIM,)).astype(np.float32)
    out = kernel(x=x, edge_index=ei, W_qkv=W_qkv, b_qkv=b_qkv,
                 W_out=W_out, b_out=b_out)
    print("kernel output:", out.shape, out.dtype, np.abs(out).max())



# revision 2
# speedup vs baseline: 65.7717x; 65.7717x over previous
"""Trainium2 Bass kernel for nn_MultiHeadAttention_71502615544564 (GNN
message-passing multi-head attention).

Math note: the reference computes
    out = segment_sum(v[dst] * attn_weights[..., None], dst)
Because v is indexed by the same dst as the segment reduction,
    out[n] = v[n] * (sum_e attn_weights[e]) = v[n] * s_n / (s_n + 1e-8)
where s_n = sum_exp[n].  For any node with in-degree >= 1, s_n is a sum
of exp values bounded below by exp(attn_min - attn_max) ~ 4e-2 for this
data, so s_n / (s_n + 1e-8) = 1 - O(3e-7): the whole attention pipeline
(q/k gathers, dots, exp, scatter) cancels out of the result.  Nodes with
in-degree 0 get out[n] = b_out exactly.  Hence

    out = x @ (Wv @ W_out) + (bv @ W_out + b_out),   in-deg 0 rows = b_out

which matches the reference to ~7e-7 in f32 (measured), ~2.4e-3 with
bf16 inputs (gate is 2e-2).  The device kernel is a node-sharded GEMM
with no gathers and no collectives; the in-degree-0 fixup is a host-side
bincount (this input has none).

Sharding: node-parallel, 6250 nodes per core, each core fully computes
its own output rows.
"""

import sys

sys.path.insert(0, "/opt/trn_rl_repo")

import ml_dtypes
import numpy as np

import concourse.bacc as bacc
import concourse.mybir as mybir
import concourse.tile as tile
from concourse.bass_utils import run_bass_kernel_spmd

P = 128
N, DIM, H, HD = 50000, 128, 8, 16
NCORES = 8
NLOC = N // NCORES            # 6250 nodes per core
NKC = (NLOC + P - 1) // P     # 49 tiles
NKR = NKC * P                 # 6272 padded rows

F32 = mybir.dt.float32
BF16 = mybir.dt.bfloat16
BF = ml_dtypes.bfloat16


def build_program():
    nc = bacc.Bacc("TRN2", target_bir_lowering=False, debug=False)

    xT = nc.dram_tensor("xT", [P, NKR], BF16, kind="ExternalInput")
    wf = nc.dram_tensor("wf", [DIM, DIM], BF16, kind="ExternalInput")
    bfv = nc.dram_tensor("bfv", [1, DIM], BF16, kind="ExternalInput")
    out_loc = nc.dram_tensor("out_loc", [P, NKC, DIM], F32, kind="ExternalOutput")

    with tile.TileContext(nc) as tc:
        with (
            tc.tile_pool(name="main", bufs=1) as pool,
            tc.tile_pool(name="ps", bufs=4, space="PSUM") as ps,
        ):
            wf_sb = pool.tile([DIM, DIM], BF16)
            nc.sync.dma_start(out=wf_sb[:], in_=wf[:])
            bf_sb = pool.tile([1, DIM], BF16)
            nc.sync.dma_start(out=bf_sb[:], in_=bfv[:])
            ones = pool.tile([1, DIM], BF16)
            nc.vector.memset(ones[:], 1.0)
            xT_sb = pool.tile([P, NKR], BF16)
            nc.sync.dma_start(out=xT_sb[:], in_=xT[:])
            out_sb = pool.tile([P, NKC, DIM], F32)
            for t in range(NKC):
                pt = ps.tile([P, DIM], F32, tag="pt")
                nc.tensor.matmul(out=pt[:], lhsT=xT_sb[:, t * P:(t + 1) * P],
                                 rhs=wf_sb[:], start=True, stop=False)
                nc.tensor.matmul(out=pt[:], lhsT=ones[:], rhs=bf_sb[:],
                                 start=False, stop=True)
                nc.vector.tensor_copy(out=out_sb[:, t, :], in_=pt[:])
            nc.sync.dma_start(out=out_loc[:], in_=out_sb[:])

    nc.compile()
    return nc


def _prep(x, edge_index, W_qkv, b_qkv, W_out, b_out):
    x = np.asarray(x, np.float32)
    W_qkv = np.asarray(W_qkv, np.float32)
    b_qkv = np.asarray(b_qkv, np.float32)
    W_out = np.asarray(W_out, np.float32)
    b_out = np.asarray(b_out, np.float32)

    # v-projection columns of the packed qkv weight (per-head layout)
    hh = np.arange(H)[:, None]
    dd = np.arange(HD)[None, :]
    cols_v = (hh * 3 * HD + 2 * HD + dd).ravel()
    Wf = (W_qkv[:, cols_v] @ W_out).astype(BF)
    bf = (b_qkv[cols_v] @ W_out + b_out).astype(BF).reshape(1, DIM)

    in_maps = []
    for c in range(NCORES):
        xl = np.zeros((P, NKR), BF)
        xl[:, :NLOC] = x[c * NLOC:(c + 1) * NLOC].astype(BF).T
        in_maps.append({"xT": xl, "wf": Wf, "bfv": bf})
    return in_maps


_PROG_CACHE = {}
TRACE = False
LAST_RESULT = None


def _install_ntff_hook():
    """Provide antenv.axon_hooks (absent in this image) so
    run_bass_kernel_spmd(trace=True) can NTFF-profile via libaxon."""
    import contextlib
    import ctypes
    import types

    if "antenv.axon_hooks" in sys.modules:
        return
    try:
        from antenv import axon_hooks  # noqa: F401
        return
    except ImportError:
        pass
    so_path = "/opt/axon/libaxon_pjrt.so"
    try:
        lib = ctypes.CDLL(so_path)
    except OSError:
        return
    if not hasattr(lib, "axon_start_nrt_profile"):
        return
    lib.axon_start_nrt_profile.argtypes = [
        ctypes.POINTER(ctypes.c_int64), ctypes.c_size_t]
    lib.axon_start_nrt_profile.restype = ctypes.c_int64
    lib.axon_stop_nrt_profile.argtypes = [ctypes.c_char_p]
    lib.axon_stop_nrt_profile.restype = ctypes.c_int64

    @contextlib.contextmanager
    def _hook(output_dir, device_ids):
        import jax
        jax.devices()
        if device_ids:
            ids = (ctypes.c_int64 * len(device_ids))(*device_ids)
            rc = lib.axon_start_nrt_profile(ids, len(device_ids))
        else:
            rc = lib.axon_start_nrt_profile(None, 0)
        if rc != 0:
            raise RuntimeError(f"axon_start_nrt_profile rc={rc}")
        try:
            yield
        finally:
            n = lib.axon_stop_nrt_profile(str(output_dir).encode())
            print(f"ntff profile: {n} file(s) -> {output_dir}", file=sys.stderr)

    _h = [_hook]
    m = types.ModuleType("antenv.axon_hooks")
    m.get_axon_ntff_profile_hook = lambda: _h[0]
    m.set_axon_ntff_profile_hook = lambda h: _h.__setitem__(0, h)
    sys.modules["antenv.axon_hooks"] = m
    import antenv
    antenv.axon_hooks = m


def kernel(x, edge_index, W_qkv, b_qkv, W_out, b_out):
    in_maps = _prep(x, edge_index, W_qkv, b_qkv, W_out, b_out)
    if "p" not in _PROG_CACHE:
        _PROG_CACHE["p"] = build_program()
    nc = _PROG_CACHE["p"]
    if TRACE:
        _install_ntff_hook()
    res = run_bass_kernel_spmd(nc, in_maps, list(range(NCORES)), trace=TRACE)
    global LAST_RESULT
    LAST_RESULT = res
    out = np.empty((N, DIM), np.float32)
    ln = np.arange(NLOC)
    pp, cc = ln % P, ln // P
    for c in range(NCORES):
        o = np.asarray(res.results[c]["out_loc"])
        out[c * NLOC:(c + 1) * NLOC] = o[pp, cc, :]

    # nodes with in-degree 0 receive no messages: out = b_out exactly
    dst = np.asarray(edge_index)[1].astype(np.int64)
    deg = np.bincount(dst, minlength=N)
    miss = deg == 0
    if miss.any():
        out[miss] = np.asarray(b_out, np.float32)
    return out


if __name__ == "__main__":
    rng = np.random.default_rng(0)
    x = rng.standard_normal((N, DIM)).astype(np.float32)
    ei = rng.integers(0, N, (2, 640000)).astype(np.int64)
    lim = 1.0 / np.sqrt(DIM)
    W_qkv = rng.uniform(-lim, lim, (DIM, 3 * DIM)).astype(np.float32)
    b_qkv = rng.uniform(-lim, lim, (3 * DIM,)).astype(np.float32)
    W_out = rng.uniform(-lim, lim, (DIM, DIM)).astype(np.float32)
    b_out = rng.uniform(-lim, lim, (DIM,)).astype(np.float32)
    out = kernel(x=x, edge_index=ei, W_qkv=W_qkv, b_qkv=b_qkv,
                 W_out=W_out, b_out=b_out)
    print("kernel output:", out.shape, out.dtype, np.abs(out).max())


# revision 3
# speedup vs baseline: 119.5335x; 1.8174x over previous
"""Trainium2 Bass kernel for nn_MultiHeadAttention_71502615544564 (GNN
message-passing multi-head attention).

Math note: the reference computes
    out = segment_sum(v[dst] * attn_weights[..., None], dst)
Because v is indexed by the same dst as the segment reduction,
    out[n] = v[n] * (sum_e attn_weights[e]) = v[n] * s_n / (s_n + 1e-8)
where s_n = sum_exp[n].  For any node with in-degree >= 1, s_n is a sum
of exp values bounded below by exp(attn_min - attn_max) ~ 4e-2 for this
data, so s_n / (s_n + 1e-8) = 1 - O(3e-7): the whole attention pipeline
(q/k gathers, dots, exp, scatter) cancels out of the result.  Nodes with
in-degree 0 get out[n] = b_out exactly.  Hence

    out = x @ (Wv @ W_out) + (bv @ W_out + b_out),   in-deg 0 rows = b_out

which matches the reference to ~7e-7 in f32 (measured), ~4.6e-4 with
fp16 inputs/outputs (gate is 2e-2).  The device kernel is a node-sharded
GEMM with no gathers and no collectives; the in-degree-0 fixup is a
host-side bincount (this input has none).

Device layout: transposed GEMM.  outT[o, n] = Wf.T @ xT with the folded
weight Wf as the stationary (lhsT) operand, nodes on the 512-wide free
axis -> 13 matmuls per core instead of 98, one LDWEIGHTS.  PSUM->SBUF
copies alternate between the Vector and Scalar engines; bias is added on
the host during unshard.

Sharding: node-parallel, 6250 nodes per core, each core fully computes
its own output rows.
"""

import sys

sys.path.insert(0, "/opt/trn_rl_repo")

import numpy as np

import concourse.bacc as bacc
import concourse.mybir as mybir
import concourse.tile as tile
from concourse.bass_utils import run_bass_kernel_spmd

P = 128
N, DIM, H, HD = 50000, 128, 8, 16
NCORES = 8
NLOC = N // NCORES            # 6250 nodes per core
NKC = (NLOC + P - 1) // P     # 49 tiles
NKR = NKC * P                 # 6272 padded rows
MM = 512                      # matmul free-dim chunk (one PSUM bank)
IN_CH = 1536                  # input DMA chunk (3 matmul chunks)
OUT_SPLIT = 3072              # first output DMA covers cols [0, 3072)

F32 = mybir.dt.float32
FP16 = mybir.dt.float16


def build_program():
    nc = bacc.Bacc("TRN2", target_bir_lowering=False, debug=False)

    xT = nc.dram_tensor("xT", [P, NKR], FP16, kind="ExternalInput")
    wf = nc.dram_tensor("wf", [DIM, DIM], FP16, kind="ExternalInput")
    out_loc = nc.dram_tensor("out_loc", [P, NKR], FP16, kind="ExternalOutput")

    with tile.TileContext(nc) as tc:
        with (
            tc.tile_pool(name="main", bufs=1) as pool,
            tc.tile_pool(name="ps", bufs=4, space="PSUM") as ps,
        ):
            wf_sb = pool.tile([DIM, DIM], FP16)
            nc.sync.dma_start(out=wf_sb[:], in_=wf[:])
            xT_sb = pool.tile([P, NKR], FP16)
            out_sb = pool.tile([P, NKR], FP16)

            # chunked input DMA so matmuls chase the transfers
            for o0 in range(0, NKR, IN_CH):
                sz = min(IN_CH, NKR - o0)
                nc.sync.dma_start(out=xT_sb[:, o0:o0 + sz], in_=xT[:, o0:o0 + sz])

            for i, m0 in enumerate(range(0, NKR, MM)):
                msz = min(MM, NKR - m0)
                pt = ps.tile([P, MM], F32, tag="pt")
                nc.tensor.matmul(out=pt[:, :msz], lhsT=wf_sb[:],
                                 rhs=xT_sb[:, m0:m0 + msz], start=True, stop=True)
                if i % 2 == 0:
                    nc.vector.tensor_copy(out=out_sb[:, m0:m0 + msz],
                                          in_=pt[:, :msz])
                else:
                    nc.scalar.activation(out=out_sb[:, m0:m0 + msz],
                                         in_=pt[:, :msz],
                                         func=mybir.ActivationFunctionType.Copy)

            nc.sync.dma_start(out=out_loc[:, :OUT_SPLIT],
                              in_=out_sb[:, :OUT_SPLIT])
            nc.sync.dma_start(out=out_loc[:, OUT_SPLIT:],
                              in_=out_sb[:, OUT_SPLIT:])

    nc.compile()
    return nc


def _prep(x, edge_index, W_qkv, b_qkv, W_out, b_out):
    x = np.asarray(x, np.float32)
    W_qkv = np.asarray(W_qkv, np.float32)
    b_qkv = np.asarray(b_qkv, np.float32)
    W_out = np.asarray(W_out, np.float32)
    b_out = np.asarray(b_out, np.float32)

    # v-projection columns of the packed qkv weight (per-head layout)
    hh = np.arange(H)[:, None]
    dd = np.arange(HD)[None, :]
    cols_v = (hh * 3 * HD + 2 * HD + dd).ravel()
    Wf = (W_qkv[:, cols_v] @ W_out).astype(np.float16)
    bf = b_qkv[cols_v] @ W_out + b_out  # f32, added on host

    in_maps = []
    for c in range(NCORES):
        xl = np.zeros((P, NKR), np.float16)
        xl[:, :NLOC] = x[c * NLOC:(c + 1) * NLOC].astype(np.float16).T
        in_maps.append({"xT": xl, "wf": Wf})
    return in_maps, bf


_PROG_CACHE = {}
TRACE = False
LAST_RESULT = None


def _install_ntff_hook():
    """Provide antenv.axon_hooks (absent in this image) so
    run_bass_kernel_spmd(trace=True) can NTFF-profile via libaxon."""
    import contextlib
    import ctypes
    import types

    if "antenv.axon_hooks" in sys.modules:
        return
    try:
        from antenv import axon_hooks  # noqa: F401
        return
    except ImportError:
        pass
    so_path = "/opt/axon/libaxon_pjrt.so"
    try:
        lib = ctypes.CDLL(so_path)
    except OSError:
        return
    if not hasattr(lib, "axon_start_nrt_profile"):
        return
    lib.axon_start_nrt_profile.argtypes = [
        ctypes.POINTER(ctypes.c_int64), ctypes.c_size_t]
    lib.axon_start_nrt_profile.restype = ctypes.c_int64
    lib.axon_stop_nrt_profile.argtypes = [ctypes.c_char_p]
    lib.axon_stop_nrt_profile.restype = ctypes.c_int64

    @contextlib.contextmanager
    def _hook(output_dir, device_ids):
        import jax
        jax.devices()
        if device_ids:
            ids = (ctypes.c_int64 * len(device_ids))(*device_ids)
            rc = lib.axon_start_nrt_profile(ids, len(device_ids))
        else:
            rc = lib.axon_start_nrt_profile(None, 0)
        if rc != 0:
            raise RuntimeError(f"axon_start_nrt_profile rc={rc}")
        try:
            yield
        finally:
            n = lib.axon_stop_nrt_profile(str(output_dir).encode())
            print(f"ntff profile: {n} file(s) -> {output_dir}", file=sys.stderr)

    _h = [_hook]
    m = types.ModuleType("antenv.axon_hooks")
    m.get_axon_ntff_profile_hook = lambda: _h[0]
    m.set_axon_ntff_profile_hook = lambda h: _h.__setitem__(0, h)
    sys.modules["antenv.axon_hooks"] = m
    import antenv
    antenv.axon_hooks = m


def kernel(x, edge_index, W_qkv, b_qkv, W_out, b_out):
    in_maps, bf = _prep(x, edge_index, W_qkv, b_qkv, W_out, b_out)
    if "p" not in _PROG_CACHE:
        _PROG_CACHE["p"] = build_program()
    nc = _PROG_CACHE["p"]
    if TRACE:
        _install_ntff_hook()
    res = run_bass_kernel_spmd(nc, in_maps, list(range(NCORES)), trace=TRACE)
    global LAST_RESULT
    LAST_RESULT = res
    out = np.empty((N, DIM), np.float32)
    for c in range(NCORES):
        o = np.asarray(res.results[c]["out_loc"])  # [DIM, NKR] fp16
        out[c * NLOC:(c + 1) * NLOC] = o[:, :NLOC].T.astype(np.float32) + bf

    # nodes with in-degree 0 receive no messages: out = b_out exactly
    dst = np.asarray(edge_index)[1].astype(np.int64)
    deg = np.bincount(dst, minlength=N)
    miss = deg == 0
    if miss.any():
        out[miss] = np.asarray(b_out, np.float32)
    return out


if __name__ == "__main__":
    rng = np.random.default_rng(0)
    x = rng.standard_normal((N, DIM)).astype(np.float32)
    ei = rng.integers(0, N, (2, 640000)).astype(np.int64)
    lim = 1.0 / np.sqrt(DIM)
    W_qkv = rng.uniform(-lim, lim, (DIM, 3 * DIM)).astype(np.float32)
    b_qkv = rng.uniform(-lim, lim, (3 * DIM,)).astype(np.float32)
    W_out = rng.uniform(-lim, lim, (DIM, DIM)).astype(np.float32)
    b_out = rng.uniform(-lim, lim, (DIM,)).astype(np.float32)
    out = kernel(x=x, edge_index=ei, W_qkv=W_qkv, b_qkv=b_qkv,
                 W_out=W_out, b_out=b_out)
    print("kernel output:", out.shape, out.dtype, np.abs(out).max())


# revision 5
# speedup vs baseline: 245.8163x; 2.0565x over previous
"""Trainium2 Bass kernel for nn_MultiHeadAttention_71502615544564 (GNN
message-passing multi-head attention).

Math note: the reference computes
    out = segment_sum(v[dst] * attn_weights[..., None], dst)
Because v is indexed by the same dst as the segment reduction,
    out[n] = v[n] * (sum_e attn_weights[e]) = v[n] * s_n / (s_n + 1e-8)
where s_n = sum_exp[n].  For any node with in-degree >= 1, s_n is a sum
of exp values bounded below by exp(attn_min - attn_max) ~ 4e-2 for this
data, so s_n / (s_n + 1e-8) = 1 - O(3e-7): the whole attention pipeline
(q/k gathers, dots, exp, scatter) cancels out of the result.  Nodes with
in-degree 0 get out[n] = b_out exactly.  Hence

    out = x @ (Wv @ W_out) + (bv @ W_out + b_out),   in-deg 0 rows = b_out

which matches the reference to ~7e-7 in f32 (measured), ~4.6e-4 with
fp16 inputs/outputs (gate is 2e-2).  The device kernel is a node-sharded
GEMM with no gathers and no collectives; the in-degree-0 fixup is a
host-side bincount (this input has none).

Device layout: transposed GEMM.  outT[o, n] = Wf.T @ xT with the folded
weight Wf (bf16, packed as the head of the input tensor) as the
stationary (lhsT) operand, nodes on the 512-wide free axis -> 13
matmuls per core.  PSUM->SBUF copies (f32 -> fp16) alternate between
the Vector and Scalar engines; bias is added on the host during
unshard.  Input DMA is chunked (aligned to matmul chunks, small first
chunk) so compute chases the transfer; output DMA is 3 chunks so the
writeback overlaps the copy pipeline.

Sharding: node-parallel, 6250 nodes per core, each core fully computes
its own output rows.
"""

import sys

sys.path.insert(0, "/opt/trn_rl_repo")

import ml_dtypes
import numpy as np

import concourse.bacc as bacc
import concourse.mybir as mybir
import concourse.tile as tile
from concourse.bass_utils import run_bass_kernel_spmd

P = 128
N, DIM, H, HD = 50000, 128, 8, 16
NCORES = 8
NLOC = N // NCORES            # 6250 nodes per core
NKC = (NLOC + P - 1) // P     # 49 tiles
NKR = NKC * P                 # 6272 padded rows
XW = DIM + NKR                # packed input: [wf | xT]
MM = 512                      # matmul free-dim chunk (one PSUM bank)
# input DMA chunks in packed-column space, aligned to matmul chunks
IN_CHUNKS = [(0, 640), (640, 2048), (2688, 2048), (4736, 1664)]
# output DMA chunks in node-column space, aligned to matmul chunks
OUT_CHUNKS = [(0, 2560), (2560, 2560), (5120, 1152)]

F32 = mybir.dt.float32
FP16 = mybir.dt.float16
BF16 = mybir.dt.bfloat16
BF = ml_dtypes.bfloat16


def build_program():
    nc = bacc.Bacc("TRN2", target_bir_lowering=False, debug=False)

    xw = nc.dram_tensor("xw", [P, XW], BF16, kind="ExternalInput")
    out_loc = nc.dram_tensor("out_loc", [P, NKR], FP16, kind="ExternalOutput")

    with tile.TileContext(nc) as tc:
        with (
            tc.tile_pool(name="main", bufs=1) as pool,
            tc.tile_pool(name="ps", bufs=4, space="PSUM") as ps,
        ):
            xw_sb = pool.tile([P, XW], BF16)
            out_sb = pool.tile([P, NKR], FP16)

            for o0, sz in IN_CHUNKS:
                nc.sync.dma_start(out=xw_sb[:, o0:o0 + sz], in_=xw[:, o0:o0 + sz])

            wf_sb = xw_sb[:, 0:DIM]
            for i, m0 in enumerate(range(0, NKR, MM)):
                msz = min(MM, NKR - m0)
                pt = ps.tile([P, MM], F32, tag="pt")
                nc.tensor.matmul(out=pt[:, :msz], lhsT=wf_sb,
                                 rhs=xw_sb[:, DIM + m0:DIM + m0 + msz],
                                 start=True, stop=True)
                if i % 2 == 0:
                    nc.vector.tensor_copy(out=out_sb[:, m0:m0 + msz],
                                          in_=pt[:, :msz])
                else:
                    nc.scalar.activation(out=out_sb[:, m0:m0 + msz],
                                         in_=pt[:, :msz],
                                         func=mybir.ActivationFunctionType.Copy)

            for o0, sz in OUT_CHUNKS:
                nc.sync.dma_start(out=out_loc[:, o0:o0 + sz],
                                  in_=out_sb[:, o0:o0 + sz])

    nc.compile()
    return nc


def _prep(x, edge_index, W_qkv, b_qkv, W_out, b_out):
    x = np.asarray(x, np.float32)
    W_qkv = np.asarray(W_qkv, np.float32)
    b_qkv = np.asarray(b_qkv, np.float32)
    W_out = np.asarray(W_out, np.float32)
    b_out = np.asarray(b_out, np.float32)

    # v-projection columns of the packed qkv weight (per-head layout)
    hh = np.arange(H)[:, None]
    dd = np.arange(HD)[None, :]
    cols_v = (hh * 3 * HD + 2 * HD + dd).ravel()
    Wf = (W_qkv[:, cols_v] @ W_out).astype(BF)
    bf = b_qkv[cols_v] @ W_out + b_out  # f32, added on host

    in_maps = []
    for c in range(NCORES):
        xl = np.zeros((P, XW), BF)
        xl[:, :DIM] = Wf
        xl[:, DIM:DIM + NLOC] = x[c * NLOC:(c + 1) * NLOC].astype(BF).T
        in_maps.append({"xw": xl})
    return in_maps, bf


_PROG_CACHE = {}
TRACE = False
LAST_RESULT = None


def _install_ntff_hook():
    """Provide antenv.axon_hooks (absent in this image) so
    run_bass_kernel_spmd(trace=True) can NTFF-profile via libaxon."""
    import contextlib
    import ctypes
    import types

    if "antenv.axon_hooks" in sys.modules:
        return
    try:
        from antenv import axon_hooks  # noqa: F401
        return
    except ImportError:
        pass
    so_path = "/opt/axon/libaxon_pjrt.so"
    try:
        lib = ctypes.CDLL(so_path)
    except OSError:
        return
    if not hasattr(lib, "axon_start_nrt_profile"):
        return
    lib.axon_start_nrt_profile.argtypes = [
        ctypes.POINTER(ctypes.c_int64), ctypes.c_size_t]
    lib.axon_start_nrt_profile.restype = ctypes.c_int64
    lib.axon_stop_nrt_profile.argtypes = [ctypes.c_char_p]
    lib.axon_stop_nrt_profile.restype = ctypes.c_int64

    @contextlib.contextmanager
    def _hook(output_dir, device_ids):
        import jax
        jax.devices()
        if device_ids:
            ids = (ctypes.c_int64 * len(device_ids))(*device_ids)
            rc = lib.axon_start_nrt_profile(ids, len(device_ids))
        else:
            rc = lib.axon_start_nrt_profile(None, 0)
        if rc != 0:
            raise RuntimeError(f"axon_start_nrt_profile rc={rc}")
        try:
            yield
        finally:
            n = lib.axon_stop_nrt_profile(str(output_dir).encode())
            print(f"ntff profile: {n} file(s) -> {output_dir}", file=sys.stderr)

    _h = [_hook]
    m = types.ModuleType("antenv.axon_hooks")
    m.get_axon_ntff_profile_hook = lambda: _h[0]
    m.set_axon_ntff_profile_hook = lambda h: _h.__setitem__(0, h)
    sys.modules["antenv.axon_hooks"] = m
    import antenv
    antenv.axon_hooks = m


def kernel(x, edge_index, W_qkv, b_qkv, W_out, b_out):
    in_maps, bf = _prep(x, edge_index, W_qkv, b_qkv, W_out, b_out)
    if "p" not in _PROG_CACHE:
        _PROG_CACHE["p"] = build_program()
    nc = _PROG_CACHE["p"]
    if TRACE:
        _install_ntff_hook()
    res = run_bass_kernel_spmd(nc, in_maps, list(range(NCORES)), trace=TRACE)
    global LAST_RESULT
    LAST_RESULT = res
    out = np.empty((N, DIM), np.float32)
    for c in range(NCORES):
        o = np.asarray(res.results[c]["out_loc"])  # [DIM, NKR] fp16
        out[c * NLOC:(c + 1) * NLOC] = o[:, :NLOC].T.astype(np.float32) + bf

    # nodes with in-degree 0 receive no messages: out = b_out exactly
    dst = np.asarray(edge_index)[1].astype(np.int64)
    deg = np.bincount(dst, minlength=N)
    miss = deg == 0
    if miss.any():
        out[miss] = np.asarray(b_out, np.float32)
    return out


if __name__ == "__main__":
    rng = np.random.default_rng(0)
    x = rng.standard_normal((N, DIM)).astype(np.float32)
    ei = rng.integers(0, N, (2, 640000)).astype(np.int64)
    lim = 1.0 / np.sqrt(DIM)
    W_qkv = rng.uniform(-lim, lim, (DIM, 3 * DIM)).astype(np.float32)
    b_qkv = rng.uniform(-lim, lim, (3 * DIM,)).astype(np.float32)
    W_out = rng.uniform(-lim, lim, (DIM, DIM)).astype(np.float32)
    b_out = rng.uniform(-lim, lim, (DIM,)).astype(np.float32)
    out = kernel(x=x, edge_index=ei, W_qkv=W_qkv, b_qkv=b_qkv,
                 W_out=W_out, b_out=b_out)
    print("kernel output:", out.shape, out.dtype, np.abs(out).max())
